# revision 13
# baseline (speedup 1.0000x reference)
"""2-layer GCN encoder on 8 Trainium2 NeuronCores (Bass/Tile).

Strategy (graph/data parallel, per sharding hint):
- Nodes are sharded contiguously across 8 cores (12500 each). Each core owns
  the aggregation (scatter) for its destination nodes; edges are assigned to
  the core owning their destination.
- Reformulation: out = dinv * (S + u) + b, with u = dinv * (x @ W),
  S_i = sum_{e: dst_e = i} u[src_e]. This removes per-edge normalization -
  only per-node dinv scaling before/after aggregation.
- Each core computes u for its node shard, then an AllGather builds the full
  u table in every core's DRAM (the "halo all-gather" of source features).
- Gather of u[src] rows (256B each) uses the SWDGE dma_gather instruction
  (1024 rows/instr limit, 4 SWDGE queues). Edges are pre-grouped by
  (dst block of 128, src range of 32768) on the host (int16 index limit);
  scatter-add is a one-hot matmul accumulated in PSUM per 128-dst block.
- Host preprocessing uses only edge_index (graph structure): degree counts,
  edge grouping, index layouts. All float math on x/W/mask runs on device.
"""

import os
import numpy as np

ABLATE = os.environ.get("KERNEL_ABLATE", "")

# ---------------------------------------------------------------- config ----
CFG = dict(
    N=100000,
    E=1600000,
    IN_CH=128,
    HID=64,
    OUT=64,
    NCORES=8,
    RANGE_W=32768,
)

P = 128
MAX_CH_PER_INSTR = 5        # 640 idxs/instr: best measured SWDGE throughput
NQUEUES = 4

_cache = {}


# ---------------------------------------------------------- host planning ---
class Plan:
    pass


def _build_plan(src, dst, cfg):
    """Group edges per (core, dst-block, src-range); equalize chunk counts
    across cores (SPMD); emit per-core idx (wrapped int16) + dstloc arrays."""
    N, NCORES = cfg["N"], cfg["NCORES"]
    SHARD = N // NCORES
    NB = (SHARD + P - 1) // P
    RW = cfg["RANGE_W"]
    ranges = list(range(0, N, RW)) + [N]
    NR = len(ranges) - 1

    core = dst // SHARD
    block = (dst - core * SHARD) // P
    rng = src // RW

    # counts[c, b, r]
    counts = np.zeros((NCORES, NB, NR), dtype=np.int64)
    np.add.at(counts, (core, block, rng), 1)
    need = (counts + P - 1) // P          # chunks needed per (c,b,r)
    nchunks = need.max(axis=0)            # [NB, NR] equalized across cores

    # chunk slots ordered range-major (each range = one contiguous stream of
    # chunks across blocks); each range stream padded to a multiple of
    # MAX_CH_PER_INSTR so instructions are uniform quanta that may span blocks.
    Q = MAX_CH_PER_INSTR
    c0 = np.zeros((NB, NR), dtype=np.int64)
    rstart = np.zeros(NR + 1, dtype=np.int64)
    acc = 0
    for r in range(NR):
        rstart[r] = acc
        for b in range(NB):
            c0[b, r] = acc
            acc += nchunks[b, r]
        acc = ((acc - rstart[r] + Q - 1) // Q * Q) + rstart[r]   # pad stream
    rstart[NR] = acc
    nch_tot = int(acc)
    iw_tot = nch_tot * (P // 16)

    # instruction table: instr i covers slots [Q*i, Q*i+Q), all in one range
    n_instr = nch_tot // Q
    instr_range = np.searchsorted(rstart[1:], np.arange(n_instr) * Q, side="right")

    # per-core data arrays
    order = np.lexsort((rng, block, core))  # sort edges by (core, block, rng)
    s_sorted, d_sorted = src[order], dst[order]
    c_sorted, b_sorted, r_sorted = core[order], block[order], rng[order]

    idx_all = np.zeros((NCORES, P, iw_tot), dtype=np.int16)
    dl_all = np.full((NCORES, P, nch_tot), -1.0, dtype=np.float32)

    # position of each edge within its (c,b,r) group
    # since sorted, compute group start indices
    keys = (c_sorted * NB + b_sorted) * NR + r_sorted
    group_start = np.zeros(len(keys), dtype=np.int64)
    new_grp = np.ones(len(keys), dtype=bool)
    new_grp[1:] = keys[1:] != keys[:-1]
    starts = np.flatnonzero(new_grp)
    group_id = np.cumsum(new_grp) - 1
    pos = np.arange(len(keys)) - starts[group_id]

    chunk = c0[b_sorted, r_sorted] + pos // P       # global chunk id
    part = pos % P                                  # partition
    loc_dst = (d_sorted - c_sorted * SHARD) - b_sorted * P   # 0..127
    loc_src = (s_sorted - r_sorted * RW).astype(np.int16)    # 0..RW-1

    dl_all[c_sorted, part, chunk] = loc_dst.astype(np.float32)
    # idx wrapped layout: within-chunk row i = part (gather row (chunk_local*128+part));
    # within the *instruction*, gathered row index i_g = (chunk - instr_chunk0)*128 + part.
    # Because instr boundaries are chunk-aligned and the wrap is i_g%16 -> partition,
    # i_g//16 -> word: (chunk*128+part) yields the same (word, partition) when offset
    # by chunk0*8 words, since 128 % 16 == 0.
    word = chunk * (P // 16) + part // 16
    wpart = part % 16
    for g in range(8):
        idx_all[c_sorted, wpart + 16 * g, word] = loc_src

    plan = Plan()
    plan.cfg = cfg
    plan.SHARD, plan.NB, plan.NR = SHARD, NB, NR
    plan.NPAD = NB * P
    plan.ranges = ranges
    plan.nchunks = nchunks
    plan.c0 = c0
    plan.nch_tot, plan.iw_tot = nch_tot, iw_tot
    plan.n_instr, plan.instr_range = n_instr, instr_range
    plan.idx_all, plan.dl_all = idx_all, dl_all
    return plan


# ---------------------------------------------------------- device build ----
def _build_bass(plan):
    import concourse.bass as bass
    import concourse.tile as tile
    from concourse import bacc, mybir
    from concourse.masks import make_identity

    cfg = plan.cfg
    N, NCORES = cfg["N"], cfg["NCORES"]
    IN_CH, HID, OUT = cfg["IN_CH"], cfg["HID"], cfg["OUT"]
    SHARD, NB, NR, NPAD = plan.SHARD, plan.NB, plan.NR, plan.NPAD
    f32, i16 = mybir.dt.float32, mybir.dt.int16
    AF = mybir.ActivationFunctionType
    ALU = mybir.AluOpType

    nc = bacc.Bacc("TRN2", target_bir_lowering=False, debug=False,
                   num_devices=NCORES, num_swdge_queues=NQUEUES)

    x_d = nc.dram_tensor("x", [NPAD, IN_CH], f32, kind="ExternalInput").ap()
    w1_d = nc.dram_tensor("w1", [IN_CH, HID], f32, kind="ExternalInput").ap()
    w2_d = nc.dram_tensor("w2", [HID, OUT], f32, kind="ExternalInput").ap()
    b1_d = nc.dram_tensor("b1b", [P, HID], f32, kind="ExternalInput").ap()
    b2_d = nc.dram_tensor("b2b", [P, OUT], f32, kind="ExternalInput").ap()
    dinv_d = nc.dram_tensor("dinvw", [P, NB], f32, kind="ExternalInput").ap()
    mask_d = nc.dram_tensor("maskp", [NPAD, HID], f32, kind="ExternalInput").ap()
    idx_d = nc.dram_tensor("gidx", [P, plan.iw_tot], i16, kind="ExternalInput").ap()
    dl_d = nc.dram_tensor("dstloc", [P, plan.nch_tot], f32, kind="ExternalInput").ap()
    out_d = nc.dram_tensor("outy", [SHARD, OUT], f32, kind="ExternalOutput").ap()

    u1shard = nc.dram_tensor("u1shard", [SHARD, HID], f32)
    u1tab = nc.dram_tensor("u1tab", [N, HID], f32, addr_space="Shared")
    u2shard = nc.dram_tensor("u2shard", [SHARD, OUT], f32)
    u2tab = nc.dram_tensor("u2tab", [N, OUT], f32, addr_space="Shared")

    with tile.TileContext(nc) as tc:
        from contextlib import ExitStack
        with ExitStack() as ctx:
            cpool = ctx.enter_context(tc.tile_pool(name="const", bufs=1))
            big = ctx.enter_context(tc.tile_pool(name="big", bufs=1))
            xpool = ctx.enter_context(tc.tile_pool(name="xp", bufs=3))
            xtpool = ctx.enter_context(tc.tile_pool(name="xtp", bufs=3))
            gat = ctx.enter_context(tc.tile_pool(name="gat", bufs=16))
            ohp = ctx.enter_context(tc.tile_pool(name="ohp", bufs=12))
            evp = ctx.enter_context(tc.tile_pool(name="evp", bufs=6))
            mp = ctx.enter_context(tc.tile_pool(name="mp", bufs=3))
            psT = ctx.enter_context(tc.tile_pool(name="psT", bufs=2, space="PSUM"))
            psU = ctx.enter_context(tc.tile_pool(name="psU", bufs=2, space="PSUM"))
            psS = ctx.enter_context(tc.tile_pool(name="psS", bufs=4, space="PSUM"))

            # constants / resident data
            ident = cpool.tile([P, P], f32)
            make_identity(nc, ident[:])
            iota_i = cpool.tile([P, P], mybir.dt.int32)
            nc.gpsimd.iota(iota_i[:], pattern=[[1, P]], base=0, channel_multiplier=0)
            iota_f = cpool.tile([P, P], f32)
            nc.vector.tensor_copy(iota_f[:], iota_i[:])

            w1t = cpool.tile([IN_CH, HID], f32)
            nc.sync.dma_start(out=w1t[:], in_=w1_d[:, :])
            w2t = cpool.tile([HID, OUT], f32)
            nc.sync.dma_start(out=w2t[:], in_=w2_d[:, :])
            b1t = cpool.tile([P, HID], f32)
            nc.sync.dma_start(out=b1t[:], in_=b1_d[:, :])
            b2t = cpool.tile([P, OUT], f32)
            nc.sync.dma_start(out=b2t[:], in_=b2_d[:, :])
            dinv_t = cpool.tile([P, NB], f32)
            nc.sync.dma_start(out=dinv_t[:], in_=dinv_d[:, :])
            idx_t = big.tile([P, plan.iw_tot], i16)
            nc.sync.dma_start(out=idx_t[:], in_=idx_d[:, :])
            dl_t = big.tile([P, plan.nch_tot], f32)
            nc.sync.dma_start(out=dl_t[:], in_=dl_d[:, :])

            u1res = big.tile([P, NB * HID], f32)
            vres = big.tile([P, NB * HID], f32)
            u2res = big.tile([P, NB * OUT], f32)

            def rows_of(b):
                return min(SHARD - b * P, P)

            # ---- phase 1: u1 = dinv * (x @ W1) for own shard ----
            for b in range(NB):
                xb = xpool.tile([P, IN_CH], f32, tag="xb")
                nc.sync.dma_start(out=xb[:], in_=x_d[b * P:(b + 1) * P, :])
                xT_ps = psT.tile([P, P], f32, tag="pst")
                nc.tensor.transpose(out=xT_ps[:], in_=xb[:], identity=ident[:])
                xT = xtpool.tile([P, P], f32, tag="xT")
                nc.scalar.activation(out=xT[:], in_=xT_ps[:], func=AF.Copy)
                u1ps = psU.tile([P, HID], f32, tag="psu")
                nc.tensor.matmul(out=u1ps[:], lhsT=xT[:], rhs=w1t[:],
                                 start=True, stop=True)
                sl = slice(b * HID, (b + 1) * HID)
                nc.vector.tensor_scalar(out=u1res[:, sl], in0=u1ps[:],
                                        scalar1=dinv_t[:, b:b + 1], scalar2=None,
                                        op0=ALU.mult)
                rw = rows_of(b)
                nc.sync.dma_start(out=u1shard[b * P:b * P + rw, :],
                                  in_=u1res[:rw, sl])

            # ---- phase 2: AllGather u1 ----
            nc.gpsimd.collective_compute(
                "AllGather", ALU.bypass,
                replica_groups=[list(range(NCORES))],
                ins=[u1shard[:, :]], outs=[u1tab[:, :]])

            # ---- aggregation layer factory ----
            def aggregate(tab, post_evac):
                """S per block via gather + one-hot matmul; post_evac(b, S_psum).

                Gather instructions are uniform MAX_CH_PER_INSTR-chunk quanta of
                the per-range chunk streams (may span dst blocks); they are
                emitted lazily when a block first consumes one of their chunks.
                """
                Q = MAX_CH_PER_INSTR
                qctr = [0]
                tiles = {}
                probe = evp.tile([P, MAX_CH_PER_INSTR], f32, tag="probe")
                nc.vector.memset(probe[:], 0.0)

                def ensure_instr(i):
                    if i in tiles:
                        return tiles[i]
                    r = int(plan.instr_range[i])
                    s0 = i * Q
                    g = gat.tile([P, Q, HID], f32, tag="g")
                    base, end = plan.ranges[r], plan.ranges[r + 1]
                    nc.gpsimd.dma_gather(
                        out_ap=g[:, :, :],
                        in_ap=tab[base:end, :],
                        idxs_ap=idx_t[:, s0 * (P // 16):(s0 + Q) * (P // 16)],
                        num_idxs=Q * P,
                        num_idxs_reg=Q * P,
                        elem_size=HID,
                        queue_num=qctr[0] % NQUEUES)
                    qctr[0] += 1
                    if ABLATE == "nomm":
                        nc.vector.tensor_tensor(
                            out=probe[:, :Q], in0=probe[:, :Q],
                            in1=g[:, :, 0], op=ALU.add)
                        tiles[i] = (g, None)
                        return tiles[i]
                    oh = ohp.tile([P, Q, P], f32, tag="oh")
                    nc.vector.tensor_tensor(
                        out=oh[:, :, :],
                        in0=dl_t[:, s0:s0 + Q, None].to_broadcast([P, Q, P]),
                        in1=iota_f[:, None, :].to_broadcast([P, Q, P]),
                        op=ALU.is_equal)
                    tiles[i] = (g, oh)
                    return tiles[i]

                for b in range(NB):
                    total_ch = int(plan.nchunks[b].sum())
                    Sps = psS.tile([P, HID], f32, tag="S")
                    if total_ch == 0 or ABLATE == "nogather":
                        nc.vector.memset(Sps[:], 0.0)
                        post_evac(b, Sps)
                        continue
                    done = 0
                    for r in range(NR):
                        for j in range(int(plan.nchunks[b, r])):
                            slot = int(plan.c0[b, r]) + j
                            g, oh = ensure_instr(slot // Q)
                            if ABLATE == "nomm":
                                done += 1
                                if done == total_ch:
                                    nc.vector.memset(Sps[:], 0.0)
                                continue
                            k = slot % Q
                            nc.tensor.matmul(out=Sps[:], lhsT=oh[:, k, :],
                                             rhs=g[:, k, :],
                                             start=(done == 0),
                                             stop=(done == total_ch - 1),
                                             skip_group_check=True)
                            done += 1
                    post_evac(b, Sps)

            # ---- phase 3: layer-1 aggregation + activation + dropout ----
            def evac1(b, Sps):
                sl = slice(b * HID, (b + 1) * HID)
                t = evp.tile([P, HID], f32, tag="t1")
                nc.vector.tensor_tensor(out=t[:], in0=Sps[:], in1=u1res[:, sl],
                                        op=ALU.add)
                nc.vector.tensor_scalar(out=t[:], in0=t[:],
                                        scalar1=dinv_t[:, b:b + 1], scalar2=None,
                                        op0=ALU.mult)
                nc.vector.tensor_tensor(out=t[:], in0=t[:], in1=b1t[:], op=ALU.add)
                nc.scalar.activation(out=t[:], in_=t[:], func=AF.Relu)
                m = mp.tile([P, HID], f32, tag="m")
                nc.sync.dma_start(out=m[:], in_=mask_d[b * P:(b + 1) * P, :])
                nc.vector.tensor_tensor(out=t[:], in0=t[:], in1=m[:], op=ALU.mult)
                nc.vector.tensor_scalar(out=vres[:, sl], in0=t[:],
                                        scalar1=dinv_t[:, b:b + 1], scalar2=None,
                                        op0=ALU.mult)

            aggregate(u1tab, evac1)

            # ---- phase 4: u2 = v @ W2 per block ----
            for b in range(NB):
                sl = slice(b * HID, (b + 1) * HID)
                slo = slice(b * OUT, (b + 1) * OUT)
                vT_ps = psT.tile([HID, P], f32, tag="pst")
                nc.tensor.transpose(out=vT_ps[:], in_=vres[:, sl], identity=ident[:])
                vT = xtpool.tile([HID, P], f32, tag="vT")
                nc.scalar.activation(out=vT[:], in_=vT_ps[:], func=AF.Copy)
                u2ps = psU.tile([P, OUT], f32, tag="psu")
                nc.tensor.matmul(out=u2ps[:], lhsT=vT[:], rhs=w2t[:],
                                 start=True, stop=True)
                nc.scalar.activation(out=u2res[:, slo], in_=u2ps[:], func=AF.Copy)
                rw = rows_of(b)
                nc.sync.dma_start(out=u2shard[b * P:b * P + rw, :],
                                  in_=u2res[:rw, slo])

            # ---- phase 5: AllGather u2 ----
            nc.gpsimd.collective_compute(
                "AllGather", ALU.bypass,
                replica_groups=[list(range(NCORES))],
                ins=[u2shard[:, :]], outs=[u2tab[:, :]])

            # ---- phase 6: layer-2 aggregation + output ----
            def evac2(b, Sps):
                slo = slice(b * OUT, (b + 1) * OUT)
                t = evp.tile([P, OUT], f32, tag="t2")
                nc.vector.tensor_tensor(out=t[:], in0=Sps[:], in1=u2res[:, slo],
                                        op=ALU.add)
                nc.vector.tensor_scalar(out=t[:], in0=t[:],
                                        scalar1=dinv_t[:, b:b + 1], scalar2=None,
                                        op0=ALU.mult)
                nc.vector.tensor_tensor(out=t[:], in0=t[:], in1=b2t[:], op=ALU.add)
                rw = rows_of(b)
                nc.sync.dma_start(out=out_d[b * P:b * P + rw, :], in_=t[:rw, :])

            aggregate(u2tab, evac2)

    nc.compile()
    return nc


# ------------------------------------------------------------- entry point --
def _prep_inputs(x, edge_index, W1, b1, W2, b2, drop_mask, plan):
    cfg = plan.cfg
    N, NCORES = cfg["N"], cfg["NCORES"]
    SHARD, NB, NPAD = plan.SHARD, plan.NB, plan.NPAD
    HID, IN_CH = cfg["HID"], cfg["IN_CH"]

    src = np.asarray(edge_index[0], dtype=np.int64)
    dst = np.asarray(edge_index[1], dtype=np.int64)
    deg = np.bincount(dst, minlength=N).astype(np.float64) + 1.0
    dinv = (1.0 / np.sqrt(deg)).astype(np.float32)

    x = np.asarray(x, dtype=np.float32)
    drop_mask = np.asarray(drop_mask, dtype=np.float32)
    b1b = np.tile(np.asarray(b1, np.float32)[None, :], (P, 1))
    b2b = np.tile(np.asarray(b2, np.float32)[None, :], (P, 1))
    W1 = np.ascontiguousarray(np.asarray(W1, np.float32))
    W2 = np.ascontiguousarray(np.asarray(W2, np.float32))

    in_maps = []
    for c in range(NCORES):
        lo, hi = c * SHARD, (c + 1) * SHARD
        xp = np.zeros((NPAD, IN_CH), np.float32)
        xp[:SHARD] = x[lo:hi]
        mk = np.zeros((NPAD, HID), np.float32)
        mk[:SHARD] = drop_mask[lo:hi]
        dw = np.ones((P, NB), np.float32)
        dshard = dinv[lo:hi]
        dpad = np.ones(NPAD, np.float32)
        dpad[:SHARD] = dshard
        dw[:, :] = dpad.reshape(NB, P).T
        in_maps.append({
            "x": xp, "w1": W1, "w2": W2, "b1b": b1b, "b2b": b2b,
            "dinvw": dw, "maskp": mk,
            "gidx": plan.idx_all[c], "dstloc": plan.dl_all[c],
        })
    return in_maps


def kernel(x, edge_index, W1, b1, W2, b2, drop_mask):
    from concourse.bass_utils import run_bass_kernel_spmd

    cfg = CFG
    src = np.asarray(edge_index[0], dtype=np.int64)
    dst = np.asarray(edge_index[1], dtype=np.int64)

    key = (src.tobytes()[:64], dst.tobytes()[:64], len(src))
    if key not in _cache:
        plan = _build_plan(src, dst, cfg)
        nc = _build_bass(plan)
        _cache[key] = (plan, nc)
    plan, nc = _cache[key]

    in_maps = _prep_inputs(x, edge_index, W1, b1, W2, b2, drop_mask, plan)
    res = run_bass_kernel_spmd(nc, in_maps, core_ids=list(range(cfg["NCORES"])))
    out = np.concatenate([res.results[c]["outy"] for c in range(cfg["NCORES"])], axis=0)
    return out.astype(np.float32)
